# revision 33
# baseline (speedup 1.0000x reference)
"""Trainium2 Bass kernel for nn_CustomConv2d: 3x3 conv, B=16, Cin=Cout=128, H=W=64.

Strategy (v7):
  - Data-parallel over batch: 8 NeuronCores x 2 images each; the (128,128,9)
    weight is replicated (host pre-transposes to [cin, tap, cout] so tap k is
    a contiguous [cin, cout] stationary-operand slice).
  - All device I/O is bf16 (host casts with RNE, output upcast on host):
    halves every DMA transfer and makes LDWEIGHTS hide completely under the
    512-cycle matmul streams (measured 215-222ns/matmul vs the 213ns
    streaming floor).  Accuracy: rel_max ~3.5e-3 vs the 2e-2 gate.
  - DMA physics (measured): a [128, n] dma_start costs >=128 packets at
    ~10ns/packet (2KB max packet per partition line) after ~1.4us of ring
    spin-up, and each engine's DMAs share ONE in-order hardware queue.  So:
      * image 0 and taps 0-1 are HOST-PACKED into one DRAM tensor with
        per-line layout [taps0-1 | x0 padded 66x66 | taps2-8]; the first DMA
        moves taps 0-1 PLUS the 10 rows block 0 needs as a single
        one-packet-per-line transfer -> first real matmul at ~9.6us.
      * remaining weights and row-chunks are paced across both rings so no
        matmul ever waits, even at full clock (sync: head, taps6-8, rows
        18-66; scalar: taps2-5, rows 10-18, then image 1).
      * stores never share a ring with loads the PE is still waiting on,
        and the ot pool is deep enough (12) that store completion never
        backpressures the matmul pipeline.
  - HAM un-throttles the PE clock 1.2->2.4 GHz after ~2.5-4.5us of array
    activity (noisy; idle gaps delay it 1:1): an unbroken burst of N=256
    warm-up matmuls on a zeroed tile runs from ~7.0us until the first real
    operands land.
  - Conv = 9 accumulating PE matmuls per 8-row output block (contraction
    over Cin=128 on the partition dim); tap (dy,dx) reads the 2D window
    [[66,8],[1,64]] at offset (y0+dy)*66 + dx (host pre-padding makes every
    tap exact, no edge fixup).
  - Output: PSUM fp32 -> SBUF bf16 cast (vector); the FINAL block runs as
    two 4-row sub-blocks each stored as two 64-partition halves (one per
    ring, 64 packets each) so the exit drain is short.
"""

import numpy as np
import ml_dtypes

import concourse.bass as bass  # noqa: F401  (registers bass types)
import concourse.tile as tile
import concourse.mybir as mybir
from concourse import bacc, bass_utils

F32 = mybir.dt.float32
BF16 = mybir.dt.bfloat16

B, CIN, COUT, KK, H, W = 16, 128, 128, 3, 64, 64
NCORES = 8
BPC = B // NCORES  # batches per core
HW = H * W         # 4096
PW = W + 2         # padded row length (66)
PH = H + 2         # padded rows (66)
XLEN = PH * PW     # 4356
ROWBLK = 8         # output rows per PSUM block (8*64=512 = one fp32 PSUM bank)
NBLK = H // ROWBLK

NTAPS = KK * KK
HEADW = 2          # taps packed before x0 in the hx tensor
X0OFF = HEADW * COUT           # 256
TAILOFF = X0OFF + XLEN         # 4612: taps 2-8 live here
HXLEN = TAILOFF + (NTAPS - HEADW) * COUT  # 5508

NWARM = 14         # N=256 warm-up matmuls (~213ns each at 1.2GHz)
HEADROWS = 10      # x0 rows folded into the first (head) DMA
TRACE = False      # set True to capture an NTFF profile (fills LAST_EXEC_NS)
LAST_EXEC_NS = None

_CACHE = {}


def _build():
    nc = bacc.Bacc("TRN2", target_bir_lowering=False, debug=False, num_devices=NCORES)
    hx_d = nc.dram_tensor("hx", [CIN, HXLEN], BF16, kind="ExternalInput").ap()
    x1_d = nc.dram_tensor("x1", [CIN, XLEN], BF16, kind="ExternalInput").ap()
    o_d = nc.dram_tensor("o", [BPC, COUT, HW], BF16, kind="ExternalOutput").ap()

    with tile.TileContext(nc) as tc:
        with (
            tc.tile_pool(name="hx", bufs=1) as hxp,
            tc.tile_pool(name="xin", bufs=1) as xp,
            tc.tile_pool(name="ps", bufs=4, space="PSUM") as pp,
            tc.tile_pool(name="ot", bufs=12) as op,
            tc.tile_pool(name="warm", bufs=1) as wmp,
            tc.tile_pool(name="warmps", bufs=1, space="PSUM") as wpp,
        ):
            # PE warm-up burst: unbroken N=256 matmuls trip the HAM clock
            # un-throttle while the first input DMAs stream, ending right as
            # the first real operands land.
            wz = wmp.tile([CIN, 2 * COUT], BF16)
            nc.gpsimd.memset(wz[:], 0.0)
            wps = wpp.tile([COUT, 2 * COUT], F32)
            for _ in range(NWARM):
                nc.tensor.matmul(wps[:], wz[:, :COUT], wz[:], start=True, stop=True)

            hx = hxp.tile([CIN, HXLEN], BF16)
            x1 = xp.tile([CIN, XLEN], BF16)

            def wsl(t, k):
                if k < HEADW:
                    return t[:, k * COUT : (k + 1) * COUT]
                return t[:, TAILOFF + (k - HEADW) * COUT : TAILOFF + (k - HEADW + 1) * COUT]

            def x0c(r0, r1):  # x0 rows [r0, r1) as hx columns
                return slice(X0OFF + PW * r0, X0OFF + PW * r1)

            def hxdma(eng, sl):
                eng.dma_start(hx[:, sl], hx_d[:, sl])

            # sync ring: head (taps 0-1 + rows 0-10, one packet per line),
            #            taps 6-8, rows 18-66; scalar: taps 2-5, rows 10-18,
            #            then image 1.
            hxdma(nc.sync, slice(0, X0OFF + PW * HEADROWS))
            hxdma(nc.scalar, slice(TAILOFF, TAILOFF + 4 * COUT))          # taps 2-5
            hxdma(nc.sync, slice(TAILOFF + 4 * COUT, HXLEN))              # taps 6-8
            hxdma(nc.scalar, x0c(HEADROWS, 18))
            hxdma(nc.sync, x0c(18, 26))
            hxdma(nc.sync, x0c(26, 34))
            hxdma(nc.sync, x0c(34, 46))
            hxdma(nc.sync, x0c(46, 58))
            hxdma(nc.sync, x0c(58, PH))
            nc.scalar.dma_start(x1[:, : PW * 22], x1_d[:, : PW * 22])
            nc.scalar.dma_start(x1[:, PW * 22 : PW * 44], x1_d[:, PW * 22 : PW * 44])
            nc.scalar.dma_start(x1[:, PW * 44 :], x1_d[:, PW * 44 :])

            xrfs = [
                hx[:, X0OFF:TAILOFF].rearrange("p (r c) -> p r c", c=PW),
                x1[:].rearrange("p (r c) -> p r c", c=PW),
            ]

            def conv_block(lb, y0, nrows):
                """nrows output rows at y0: 9 accumulating matmuls -> PSUM."""
                n = nrows * W
                ps = pp.tile([COUT, n], F32)
                first = True
                for dy in range(KK):
                    for dx in range(KK):
                        nc.tensor.matmul(
                            ps[:],
                            wsl(hx, dy * KK + dx),
                            xrfs[lb][:, y0 + dy : y0 + dy + nrows, dx : dx + W],
                            start=first,
                            stop=(dy == KK - 1 and dx == KK - 1),
                        )
                        first = False
                return ps

            half = COUT // 2
            for lb in range(BPC):
                for yb in range(NBLK):
                    y0 = yb * ROWBLK
                    gi = lb * NBLK + yb
                    last = lb == BPC - 1 and yb == NBLK - 1
                    if last:
                        # final block as two 4-row sub-blocks; each cast then
                        # stored as two 64-partition halves, one per ring, so
                        # the exit drain is short parallel transfers
                        for h_ in range(2):
                            yh = y0 + h_ * (ROWBLK // 2)
                            n = (ROWBLK // 2) * W
                            ps = conv_block(lb, yh, ROWBLK // 2)
                            ot = op.tile([COUT, n], BF16)
                            nc.vector.tensor_copy(ot[:], ps[:])
                            nc.scalar.dma_start(
                                o_d[lb][:half, W * yh : W * yh + n], ot[:half, :]
                            )
                            nc.sync.dma_start(
                                o_d[lb][half:, W * yh : W * yh + n], ot[half:, :]
                            )
                    else:
                        ps = conv_block(lb, y0, ROWBLK)
                        ot = op.tile([COUT, ROWBLK * W], BF16)
                        nc.vector.tensor_copy(ot[:], ps[:])
                        st_eng = nc.scalar if gi % 2 == 0 else nc.sync
                        st_eng.dma_start(
                            o_d[lb][:, W * y0 : W * y0 + ROWBLK * W], ot[:]
                        )
    nc.compile()
    return nc


def _get_nc():
    if "nc" not in _CACHE:
        _CACHE["nc"] = _build()
    return _CACHE["nc"]


def kernel(x, weights):
    """x: [16,128,64,64] f32; weights: [128,128,9] f32 -> [2048,64,64] f32."""
    global LAST_EXEC_NS
    x = np.asarray(x, dtype=np.float32)
    w = np.asarray(weights, dtype=np.float32)
    # [cout, cin, k] -> [cin, k, cout] so tap k is a contiguous lhsT slice
    wT = np.ascontiguousarray(w.transpose(1, 2, 0)).reshape(CIN, NTAPS * COUT)
    xpad = np.zeros((B, CIN, PH, PW), np.float32)
    xpad[:, :, 1 : H + 1, 1 : W + 1] = x
    wT16 = wT.astype(ml_dtypes.bfloat16)
    xpad16 = xpad.reshape(B, CIN, XLEN).astype(ml_dtypes.bfloat16)

    nc = _get_nc()
    xr = xpad16.reshape(NCORES, BPC, CIN, XLEN)
    in_maps = []
    for c in range(NCORES):
        hx = np.concatenate(
            [wT16[:, : HEADW * COUT], xr[c, 0], wT16[:, HEADW * COUT :]], axis=1
        )
        in_maps.append(
            {"hx": np.ascontiguousarray(hx), "x1": np.ascontiguousarray(xr[c, 1])}
        )

    res = bass_utils.run_bass_kernel_spmd(
        nc, in_maps, core_ids=list(range(NCORES)), trace=TRACE
    )
    LAST_EXEC_NS = res.exec_time_ns

    arr = np.stack([res.results[c]["o"] for c in range(NCORES)])  # [8, 2, 128, 4096]
    arr = arr.astype(np.float32)
    # out[cout*B + b] = conv[b, cout], with b = core*BPC + lb
    arr = arr.transpose(2, 0, 1, 3).reshape(COUT, B, H, W)
    return np.ascontiguousarray(arr.reshape(COUT * B, H, W))


# revision 34
# speedup vs baseline: 1.0221x; 1.0221x over previous
"""Trainium2 Bass kernel for nn_CustomConv2d: 3x3 conv, B=16, Cin=Cout=128, H=W=64.

Strategy (v7):
  - Data-parallel over batch: 8 NeuronCores x 2 images each; the (128,128,9)
    weight is replicated (host pre-transposes to [cin, tap, cout] so tap k is
    a contiguous [cin, cout] stationary-operand slice).
  - All device I/O is bf16 (host casts with RNE, output upcast on host):
    halves every DMA transfer and makes LDWEIGHTS hide completely under the
    512-cycle matmul streams (measured 215-222ns/matmul vs the 213ns
    streaming floor).  Accuracy: rel_max ~3.5e-3 vs the 2e-2 gate.
  - DMA physics (measured): a [128, n] dma_start costs >=128 packets at
    ~10ns/packet (2KB max packet per partition line) after ~1.4us of ring
    spin-up, and each engine's DMAs share ONE in-order hardware queue.  So:
      * image 0 and taps 0-1 are HOST-PACKED into one DRAM tensor with
        per-line layout [taps0-1 | x0 padded 66x66 | taps2-8]; the first DMA
        moves taps 0-1 PLUS the 10 rows block 0 needs as a single
        one-packet-per-line transfer -> first real matmul at ~9.6us.
      * remaining weights and row-chunks are paced across both rings so no
        matmul ever waits, even at full clock (sync: head, taps6-8, rows
        18-66; scalar: taps2-5, rows 10-18, then image 1).
      * stores never share a ring with loads the PE is still waiting on,
        and the ot pool is deep enough (12) that store completion never
        backpressures the matmul pipeline.
  - HAM un-throttles the PE clock 1.2->2.4 GHz after ~2.5-4.5us of array
    activity (noisy; idle gaps delay it 1:1): an unbroken burst of N=256
    warm-up matmuls on a zeroed tile runs from ~7.0us until the first real
    operands land.
  - Conv = 9 accumulating PE matmuls per 8-row output block (contraction
    over Cin=128 on the partition dim); tap (dy,dx) reads the 2D window
    [[66,8],[1,64]] at offset (y0+dy)*66 + dx (host pre-padding makes every
    tap exact, no edge fixup).
  - Output: PSUM fp32 -> SBUF bf16 cast (vector); the FINAL block runs as
    two 4-row sub-blocks each stored as two 64-partition halves (one per
    ring, 64 packets each) so the exit drain is short.
"""

import numpy as np
import ml_dtypes

import concourse.bass as bass  # noqa: F401  (registers bass types)
import concourse.tile as tile
import concourse.mybir as mybir
from concourse import bacc, bass_utils

F32 = mybir.dt.float32
BF16 = mybir.dt.bfloat16

B, CIN, COUT, KK, H, W = 16, 128, 128, 3, 64, 64
NCORES = 8
BPC = B // NCORES  # batches per core
HW = H * W         # 4096
PW = W + 2         # padded row length (66)
PH = H + 2         # padded rows (66)
XLEN = PH * PW     # 4356
ROWBLK = 8         # output rows per PSUM block (8*64=512 = one fp32 PSUM bank)
NBLK = H // ROWBLK

NTAPS = KK * KK
HEADW = 2          # taps packed before x0 in the hx tensor
X0OFF = HEADW * COUT           # 256
TAILOFF = X0OFF + XLEN         # 4612: taps 2-8 live here
HXLEN = TAILOFF + (NTAPS - HEADW) * COUT  # 5508

NWARM = 12         # N=256 warm-up matmuls (~213ns each at 1.2GHz)
HEADROWS = 10      # x0 rows folded into the first (head) DMA
TRACE = False      # set True to capture an NTFF profile (fills LAST_EXEC_NS)
LAST_EXEC_NS = None

_CACHE = {}


def _build():
    nc = bacc.Bacc("TRN2", target_bir_lowering=False, debug=False, num_devices=NCORES)
    hx_d = nc.dram_tensor("hx", [CIN, HXLEN], BF16, kind="ExternalInput").ap()
    x1_d = nc.dram_tensor("x1", [CIN, XLEN], BF16, kind="ExternalInput").ap()
    o_d = nc.dram_tensor("o", [BPC, COUT, HW], BF16, kind="ExternalOutput").ap()

    with tile.TileContext(nc) as tc:
        with (
            tc.tile_pool(name="hx", bufs=1) as hxp,
            tc.tile_pool(name="xin", bufs=1) as xp,
            tc.tile_pool(name="ps", bufs=4, space="PSUM") as pp,
            tc.tile_pool(name="ot", bufs=12) as op,
            tc.tile_pool(name="warm", bufs=1) as wmp,
            tc.tile_pool(name="warmps", bufs=1, space="PSUM") as wpp,
        ):
            # PE warm-up burst: unbroken N=256 matmuls trip the HAM clock
            # un-throttle while the first input DMAs stream, ending right as
            # the first real operands land.
            wz = wmp.tile([CIN, 2 * COUT], BF16)
            nc.gpsimd.memset(wz[:], 0.0)
            wps = wpp.tile([COUT, 2 * COUT], F32)
            for _ in range(NWARM):
                nc.tensor.matmul(wps[:], wz[:, :COUT], wz[:], start=True, stop=True)

            hx = hxp.tile([CIN, HXLEN], BF16)
            x1 = xp.tile([CIN, XLEN], BF16)

            def wsl(t, k):
                if k < HEADW:
                    return t[:, k * COUT : (k + 1) * COUT]
                return t[:, TAILOFF + (k - HEADW) * COUT : TAILOFF + (k - HEADW + 1) * COUT]

            def x0c(r0, r1):  # x0 rows [r0, r1) as hx columns
                return slice(X0OFF + PW * r0, X0OFF + PW * r1)

            def hxdma(eng, sl):
                eng.dma_start(hx[:, sl], hx_d[:, sl])

            # sync ring: head (taps 0-1 + rows 0-10, one packet per line),
            #            taps 6-8, rows 18-66; scalar: taps 2-5, rows 10-18,
            #            then image 1.
            hxdma(nc.sync, slice(0, X0OFF + PW * HEADROWS))
            hxdma(nc.scalar, slice(TAILOFF, TAILOFF + 4 * COUT))          # taps 2-5
            hxdma(nc.sync, slice(TAILOFF + 4 * COUT, HXLEN))              # taps 6-8
            hxdma(nc.scalar, x0c(HEADROWS, 18))
            hxdma(nc.sync, x0c(18, 26))
            hxdma(nc.sync, x0c(26, 34))
            hxdma(nc.sync, x0c(34, 46))
            hxdma(nc.sync, x0c(46, 58))
            hxdma(nc.sync, x0c(58, PH))
            nc.scalar.dma_start(x1[:, : PW * 22], x1_d[:, : PW * 22])
            nc.scalar.dma_start(x1[:, PW * 22 : PW * 44], x1_d[:, PW * 22 : PW * 44])
            nc.scalar.dma_start(x1[:, PW * 44 :], x1_d[:, PW * 44 :])

            xrfs = [
                hx[:, X0OFF:TAILOFF].rearrange("p (r c) -> p r c", c=PW),
                x1[:].rearrange("p (r c) -> p r c", c=PW),
            ]

            def conv_block(lb, y0, nrows):
                """nrows output rows at y0: 9 accumulating matmuls -> PSUM."""
                n = nrows * W
                ps = pp.tile([COUT, n], F32)
                first = True
                for dy in range(KK):
                    for dx in range(KK):
                        nc.tensor.matmul(
                            ps[:],
                            wsl(hx, dy * KK + dx),
                            xrfs[lb][:, y0 + dy : y0 + dy + nrows, dx : dx + W],
                            start=first,
                            stop=(dy == KK - 1 and dx == KK - 1),
                        )
                        first = False
                return ps

            half = COUT // 2
            for lb in range(BPC):
                for yb in range(NBLK):
                    y0 = yb * ROWBLK
                    gi = lb * NBLK + yb
                    last = lb == BPC - 1 and yb == NBLK - 1
                    if last:
                        # final block as two 4-row sub-blocks; each cast then
                        # stored as two 64-partition halves, one per ring, so
                        # the exit drain is short parallel transfers
                        for h_ in range(2):
                            yh = y0 + h_ * (ROWBLK // 2)
                            n = (ROWBLK // 2) * W
                            ps = conv_block(lb, yh, ROWBLK // 2)
                            ot = op.tile([COUT, n], BF16)
                            nc.vector.tensor_copy(ot[:], ps[:])
                            nc.scalar.dma_start(
                                o_d[lb][:half, W * yh : W * yh + n], ot[:half, :]
                            )
                            nc.sync.dma_start(
                                o_d[lb][half:, W * yh : W * yh + n], ot[half:, :]
                            )
                    else:
                        ps = conv_block(lb, y0, ROWBLK)
                        ot = op.tile([COUT, ROWBLK * W], BF16)
                        nc.vector.tensor_copy(ot[:], ps[:])
                        st_eng = nc.scalar if gi % 2 == 0 else nc.sync
                        st_eng.dma_start(
                            o_d[lb][:, W * y0 : W * y0 + ROWBLK * W], ot[:]
                        )
    nc.compile()
    return nc


def _get_nc():
    if "nc" not in _CACHE:
        _CACHE["nc"] = _build()
    return _CACHE["nc"]


def kernel(x, weights):
    """x: [16,128,64,64] f32; weights: [128,128,9] f32 -> [2048,64,64] f32."""
    global LAST_EXEC_NS
    x = np.asarray(x, dtype=np.float32)
    w = np.asarray(weights, dtype=np.float32)
    # [cout, cin, k] -> [cin, k, cout] so tap k is a contiguous lhsT slice
    wT = np.ascontiguousarray(w.transpose(1, 2, 0)).reshape(CIN, NTAPS * COUT)
    xpad = np.zeros((B, CIN, PH, PW), np.float32)
    xpad[:, :, 1 : H + 1, 1 : W + 1] = x
    wT16 = wT.astype(ml_dtypes.bfloat16)
    xpad16 = xpad.reshape(B, CIN, XLEN).astype(ml_dtypes.bfloat16)

    nc = _get_nc()
    xr = xpad16.reshape(NCORES, BPC, CIN, XLEN)
    in_maps = []
    for c in range(NCORES):
        hx = np.concatenate(
            [wT16[:, : HEADW * COUT], xr[c, 0], wT16[:, HEADW * COUT :]], axis=1
        )
        in_maps.append(
            {"hx": np.ascontiguousarray(hx), "x1": np.ascontiguousarray(xr[c, 1])}
        )

    res = bass_utils.run_bass_kernel_spmd(
        nc, in_maps, core_ids=list(range(NCORES)), trace=TRACE
    )
    LAST_EXEC_NS = res.exec_time_ns

    arr = np.stack([res.results[c]["o"] for c in range(NCORES)])  # [8, 2, 128, 4096]
    arr = arr.astype(np.float32)
    # out[cout*B + b] = conv[b, cout], with b = core*BPC + lb
    arr = arr.transpose(2, 0, 1, 3).reshape(COUT, B, H, W)
    return np.ascontiguousarray(arr.reshape(COUT * B, H, W))
